# revision 5
# baseline (speedup 1.0000x reference)
"""GQA attention (B=2, T=2048, D=2048, H=16, HK=4, HD=128) on 8 TRN2 NeuronCores.

v4: single-blob wire format. Core c in 0..7 handles kv group g = c//2 and
q-head pair p = c%2 (heads 4g+2p, 4g+2p+1) for BOTH batches. Each core
receives ONE flat bf16 blob (~4.6MB):
  [x rows c*512:(c+1)*512 of stacked [2T, D] (natural layout) |
   rows c*32.. of [cos_e; sin_e] | wq cols for its 2 heads |
   half of its group's wk | half of wv | wo rows for its 2 heads]
On device: each core transposes its own x chunk with hardware xbar transpose
DMAs (so the host never transposes x), then an AllGather(8) of the transposed
chunks lands each (batch, q-block)'s x^T block contiguously; cos/sin gather
over 8, wk/wv over core pairs. Per-core partials (2 heads, both batches,
fp32) are summed with
a ReduceScatter(add) over all 8 cores; core c returns rows [c*512:(c+1)*512]
of the stacked [2T, D] output in bf16 (~2MB/core up).

Device pipeline per (batch, 512-wide q block): project k/v/q with RoPE (PE
matmuls, fp32 PSUM), causal attention via exp (no max subtraction; 1/sqrt(d)
folded into the activation scale), out-projection accumulated in fp32.
Constants (RoPE pair-swap matrix, identity, causal block masks) are inlined
into the NEFF. The host runner caches the compiled jit across calls and keeps
the donation-placeholder output buffers resident on device.
"""

import sys

if "/opt/trn_rl_repo" not in sys.path:
    sys.path.insert(0, "/opt/trn_rl_repo")

from contextlib import ExitStack

import ml_dtypes
import numpy as np

import concourse.bacc as bacc
import concourse.tile as tile
from concourse import mybir

BF = ml_dtypes.bfloat16

B, T, D = 2, 2048, 2048
H, HK, HD = 16, 4, 128
HPC = 2        # q heads per core
P = 128
KC = D // P    # contraction chunks for the projections
NT = T // P    # 128-row tiles of T
NQB = T // 512 # 512-wide q blocks
NC8 = 8
SCALE = float(1.0 / np.sqrt(HD))

# blob layout (elements, bf16)
OX = 0
OCS = OX + 512 * T            # 1048576
OWQ = OCS + 32 * T            # 1114112
OWK = OWQ + D * HPC * HD      # 1638400
OWV = OWK + 1024 * HD         # 1769472
OWO = OWV + 1024 * HD         # 1900544
NBLOB = OWO + HPC * HD * D    # 2424832
OMK = NBLOB                   # noncausal only
NBLOB_NC = OMK + 256 * T      # 2949120


def _consts():
    mt = np.zeros((P, P), np.float32)
    for i in range(P // 2):
        mt[2 * i + 1, 2 * i] = -1.0  # shuf[2i]   = -q[2i+1]
        mt[2 * i, 2 * i + 1] = 1.0   # shuf[2i+1] = +q[2i]
    idn = np.eye(P, dtype=np.float32)
    s_i = np.arange(P)[:, None]
    q_i = np.arange(512)[None, :]
    m_r = np.stack([(r * P + s_i <= q_i) for r in range(4)], axis=1)
    masks = np.ascontiguousarray(m_r.reshape(P, 4 * 512)).astype(np.float32)
    return mt, idn, masks


def _build(causal: bool):
    bf = mybir.dt.bfloat16
    f32 = mybir.dt.float32
    nc = bacc.Bacc("TRN2", target_bir_lowering=False, debug=False,
                   enable_asserts=False, num_devices=NC8)

    nblob = NBLOB if causal else NBLOB_NC
    blob = nc.dram_tensor("blob", [1, nblob], bf, kind="ExternalInput").ap()
    out = nc.dram_tensor("out", [512, D], bf, kind="ExternalOutput").ap()

    mt_h, idn_h, masks_h = _consts()
    mt_c = nc.inline_tensor(mt_h.astype(BF), name="mtc")
    idn_c = nc.inline_tensor(idn_h.astype(BF), name="idnc")
    if causal:
        masks_c = nc.inline_tensor(masks_h.astype(BF), name="masksc")

    # collective buffers
    # xTg8: AllGather of per-core pre-transposed x chunks. Chunk c (= b*4+qb)
    # holds x[b, qb*512:(qb+1)*512, :]^T as [D, 512], so rows
    # [c*D:(c+1)*D] are the xT block for that (b, qb).
    xTg8 = nc.dram_tensor("xTg8", [NC8 * D, 512], bf, addr_space="Shared").ap()
    csful = nc.dram_tensor("csful", [256, T], bf, addr_space="Shared").ap()
    wkful = nc.dram_tensor("wkful", [D, HD], bf).ap()
    wvful = nc.dram_tensor("wvful", [D, HD], bf).ap()
    if not causal:
        mTful = nc.dram_tensor("mTful", [T, T], bf, addr_space="Shared").ap()
    partial = nc.dram_tensor("partial", [B * T, D], f32).ap()

    EXP = mybir.ActivationFunctionType.Exp
    G8 = [list(range(NC8))]
    GP = [[0, 1], [2, 3], [4, 5], [6, 7]]

    with tile.TileContext(nc) as tc, ExitStack() as ctx:
        dram = ctx.enter_context(tc.tile_pool(name="dram", bufs=1, space="DRAM"))
        singles = ctx.enter_context(tc.tile_pool(name="singles", bufs=1))
        ps = ctx.enter_context(tc.tile_pool(name="ps", bufs=8, space="PSUM"))
        sb_x = ctx.enter_context(tc.tile_pool(name="xblk", bufs=2))
        sb_raw = ctx.enter_context(tc.tile_pool(name="raw", bufs=3))
        sb_tmp = ctx.enter_context(tc.tile_pool(name="tmp", bufs=4))
        sb_probs = ctx.enter_context(tc.tile_pool(name="probs", bufs=8))
        sb_small = ctx.enter_context(tc.tile_pool(name="small", bufs=4))
        sb_out = ctx.enter_context(tc.tile_pool(name="outst", bufs=3))
        sb_fin = ctx.enter_context(tc.tile_pool(name="fin", bufs=2))
        if not causal:
            sb_mask = ctx.enter_context(tc.tile_pool(name="mask", bufs=18))

        # ---- bounce ExternalInput slices into Local DRAM, then gather ----
        # transpose this core's own x chunk [512, D] -> [D, 512] via xbar
        # transpose DMAs (blob is an ExternalInput, so no producer hazard),
        # park it in local DRAM as the AllGather input.
        xsrc = blob[0, OX:OCS].rearrange("(a b) -> a b", a=512)  # [512, D]
        xtb = dram.tile([D, 512], bf, tag="xtb")
        for c in range(KC):
            st = sb_x.tile([P, 512], bf, tag="xstage", name=f"xst{c}")
            eng = nc.sync if c % 2 == 0 else nc.scalar
            eng.dma_start_transpose(st, xsrc[:, c * P:(c + 1) * P])
            eng.dma_start(out=xtb[c * P:(c + 1) * P, :], in_=st)
        cb = dram.tile([32, T], bf, tag="cb")
        nc.gpsimd.dma_start(
            out=cb[:], in_=blob[0, OCS:OWQ].rearrange("(a b) -> a b", a=32))
        kb = dram.tile([1024, HD], bf, tag="kb")
        nc.gpsimd.dma_start(
            out=kb[:], in_=blob[0, OWK:OWV].rearrange("(a b) -> a b", a=1024))
        vb = dram.tile([1024, HD], bf, tag="vb")
        nc.gpsimd.dma_start(
            out=vb[:], in_=blob[0, OWV:OWO].rearrange("(a b) -> a b", a=1024))
        nc.gpsimd.collective_compute(
            "AllGather", mybir.AluOpType.bypass, replica_groups=G8,
            ins=[xtb[:].opt()], outs=[xTg8.opt()])
        nc.gpsimd.collective_compute(
            "AllGather", mybir.AluOpType.bypass, replica_groups=G8,
            ins=[cb[:].opt()], outs=[csful.opt()])
        nc.gpsimd.collective_compute(
            "AllGather", mybir.AluOpType.bypass, replica_groups=GP,
            ins=[kb[:].opt()], outs=[wkful.opt()])
        nc.gpsimd.collective_compute(
            "AllGather", mybir.AluOpType.bypass, replica_groups=GP,
            ins=[vb[:].opt()], outs=[wvful.opt()])
        if not causal:
            mb = dram.tile([256, T], bf, tag="mb")
            nc.gpsimd.dma_start(
                out=mb[:], in_=blob[0, OMK:NBLOB_NC].rearrange(
                    "(a b) -> a b", a=256))
            nc.gpsimd.collective_compute(
                "AllGather", mybir.AluOpType.bypass, replica_groups=G8,
                ins=[mb[:].opt()], outs=[mTful.opt()])

        # ---- resident weights / consts ----
        wq_sb = singles.tile([P, KC, HPC * HD], bf, tag="wq")
        nc.scalar.dma_start(
            out=wq_sb, in_=blob[0, OWQ:OWK].rearrange(
                "(c p n) -> p c n", p=P, n=HPC * HD))
        wo_sb = singles.tile([P, HPC, D], bf, tag="wo")
        nc.scalar.dma_start(
            out=wo_sb, in_=blob[0, OWO:NBLOB].rearrange(
                "(h p d) -> p h d", p=P, d=D))
        wk_sb = singles.tile([P, KC, HD], bf, tag="wk")
        nc.gpsimd.dma_start(out=wk_sb,
                            in_=wkful.rearrange("(c p) n -> p c n", p=P))
        wv_sb = singles.tile([P, KC, HD], bf, tag="wv")
        nc.gpsimd.dma_start(out=wv_sb,
                            in_=wvful.rearrange("(c p) n -> p c n", p=P))
        mt_sb = singles.tile([P, P], bf, tag="mt")
        nc.sync.dma_start(out=mt_sb, in_=mt_c.ap())
        id_sb = singles.tile([P, P], bf, tag="idn")
        nc.scalar.dma_start(out=id_sb, in_=idn_c.ap())
        if causal:
            masks_sb = singles.tile([P, 4, 512], bf, tag="masks")
            nc.scalar.dma_start(out=masks_sb, in_=masks_c.ap().rearrange(
                "p (r n) -> p r n", r=4))
        cos_sb = singles.tile([P, T], bf, tag="cos")
        nc.gpsimd.dma_start(out=cos_sb, in_=csful[0:P, :])
        sin_sb = singles.tile([P, T], bf, tag="sin")
        nc.gpsimd.dma_start(out=sin_sb, in_=csful[P:2 * P, :])

        qT = singles.tile([P, HPC, T], bf, tag="qT")
        kT = singles.tile([P, T], bf, tag="kT")
        vax = singles.tile([P, NT, HD + 1], bf, tag="vax")
        oT = singles.tile([P, HPC, T], bf, tag="oT")
        nc.vector.memset(vax[:, :, HD], 1.0)

        def proj_rope(dst_slice, lhsT_of, xt, nb, tag):
            sl = slice(nb * 512, (nb + 1) * 512)
            pt = ps.tile([P, 512], f32, tag="ps", name=f"pjps{tag}{nb}")
            for c in range(KC):
                nc.tensor.matmul(pt, lhsT=lhsT_of(c), rhs=xt[:, c, :],
                                 start=(c == 0), stop=(c == KC - 1))
            raw = sb_raw.tile([P, 512], bf, tag="raw", name=f"raw{tag}{nb}")
            if tag in ("k", "q1"):
                nc.scalar.copy(raw, pt)
            else:
                nc.vector.tensor_copy(raw, pt)
            sh = ps.tile([P, 512], f32, tag="ps", name=f"shps{tag}{nb}")
            nc.tensor.matmul(sh, lhsT=mt_sb, rhs=raw, start=True, stop=True)
            ta = sb_tmp.tile([P, 512], bf, tag="tmp", name=f"ta{tag}{nb}")
            nc.vector.tensor_mul(ta, raw, cos_sb[:, sl])
            tb = sb_tmp.tile([P, 512], bf, tag="tmp", name=f"tb{tag}{nb}")
            nc.vector.tensor_mul(tb, sh, sin_sb[:, sl])
            nc.vector.tensor_add(dst_slice, ta, tb)

        # ---- pipeline over (batch, 512-wide q block) ----
        # causal: projections for a block fuse with its attention (later
        # blocks' k/v are never read thanks to the causal structure).
        # noncausal: an arbitrary mask may attend to ANY position, so all of
        # the batch's k/v/q must be projected before any attention runs.
        def proj_block(b, qb):
            qsl = slice(qb * 512, (qb + 1) * 512)
            # x^T block for this (b, qb) = gathered chunk b*4+qb
            xt = sb_x.tile([P, KC, 512], bf, tag="xt", name=f"xt{b}_{qb}")
            cg = b * NQB + qb
            nc.gpsimd.dma_start(
                out=xt, in_=xTg8[cg * D:(cg + 1) * D, :].rearrange(
                    "(c p) n -> p c n", p=P))
            proj_rope(kT[:, qsl], lambda c: wk_sb[:, c], xt, qb, "k")
            for mi in range(4):
                m = qb * 4 + mi
                pv = ps.tile([P, P], f32, tag="ps",
                             name=f"vps{b}_{qb}_{mi}")
                for c in range(KC):
                    nc.tensor.matmul(pv, lhsT=xt[:, c, mi * P:(mi + 1) * P],
                                     rhs=wv_sb[:, c],
                                     start=(c == 0), stop=(c == KC - 1))
                nc.vector.tensor_copy(vax[:, m, :HD], pv)
            for h in range(HPC):
                proj_rope(qT[:, h, qsl],
                          lambda c, h=h: wq_sb[:, c, h * HD:(h + 1) * HD],
                          xt, qb, f"q{h}")

        def attn_out_block(b, qb):
                qsl = slice(qb * 512, (qb + 1) * 512)
                nj = 4 * qb + 4 if causal else NT
                if not causal:
                    mts = []
                    for j in range(nj):
                        t_ = sb_mask.tile([P, 512], bf, tag="maskt",
                                          name=f"mk{b}_{qb}_{j}")
                        nc.gpsimd.dma_start(
                            out=t_, in_=mTful[j * P:(j + 1) * P, qsl])
                        mts.append(t_)
                for h in range(HPC):
                    oaug = [ps.tile([P, HD + 1], f32, tag="ps",
                                    name=f"oa{b}_{qb}_{h}_{k}")
                            for k in range(4)]
                    for j in range(nj):
                        r = j - 4 * qb if causal else -1
                        q0 = max(r, 0) * P
                        sc = ps.tile([P, 512], f32, tag="ps",
                                     name=f"sc{b}_{qb}_{h}_{j}")
                        nc.tensor.matmul(
                            sc[:, q0:], lhsT=kT[:, j * P:(j + 1) * P],
                            rhs=qT[:, h, qb * 512 + q0:(qb + 1) * 512],
                            start=True, stop=True)
                        if not causal:
                            nc.vector.tensor_add(sc, sc, mts[j])
                        pr = sb_probs.tile([P, 512], bf, tag="probs",
                                           name=f"pr{b}_{qb}_{h}_{j}")
                        nc.scalar.activation(pr[:, q0:], sc[:, q0:], EXP,
                                             scale=SCALE)
                        if causal and r >= 0:
                            nc.vector.tensor_mul(pr[:, q0:], pr[:, q0:],
                                                 masks_sb[:, r, q0:])
                        for mi in range(4):
                            m = qb * 4 + mi
                            if causal and j > m:
                                continue
                            last = (j == m) if causal else (j == nj - 1)
                            nc.tensor.matmul(oaug[mi],
                                             lhsT=pr[:, mi * P:(mi + 1) * P],
                                             rhs=vax[:, j, :],
                                             start=(j == 0), stop=last)
                    for mi in range(4):
                        m = qb * 4 + mi
                        rec = sb_small.tile([P, 1], f32, tag="rec",
                                            name=f"rc{b}_{qb}_{h}_{mi}")
                        nc.vector.reciprocal(rec, oaug[mi][:, HD:HD + 1])
                        on = sb_small.tile([P, HD], bf, tag="onrm",
                                           name=f"on{b}_{qb}_{h}_{mi}")
                        nc.vector.tensor_scalar_mul(on, oaug[mi][:, :HD], rec)
                        tp = ps.tile([P, P], bf, tag="ps",
                                     name=f"tp{b}_{qb}_{h}_{mi}")
                        nc.tensor.transpose(tp, on, id_sb)
                        nc.vector.tensor_copy(oT[:, h, m * P:(m + 1) * P], tp)

                # -- out-projection: 4 row-tiles, fp32 into the RS buffer --
                for mi in range(4):
                    m = qb * 4 + mi
                    ost = sb_out.tile([P, D], f32, tag="outst",
                                      name=f"ost{b}_{m}")
                    for n in range(D // 512):
                        wops = ps.tile([P, 512], f32, tag="ps",
                                       name=f"wops{b}_{m}_{n}")
                        for h in range(HPC):
                            nc.tensor.matmul(
                                wops, lhsT=oT[:, h, m * P:(m + 1) * P],
                                rhs=wo_sb[:, h, n * 512:(n + 1) * 512],
                                start=(h == 0), stop=(h == HPC - 1))
                        if n == 3:
                            nc.scalar.copy(ost[:, n * 512:(n + 1) * 512], wops)
                        else:
                            nc.vector.tensor_copy(
                                ost[:, n * 512:(n + 1) * 512], wops)
                    eng = nc.sync if m % 2 == 0 else nc.scalar
                    eng.dma_start(
                        out=partial[b * T + m * P:b * T + (m + 1) * P, :],
                        in_=ost)

        for b in range(B):
            if causal:
                for qb in range(NQB):
                    proj_block(b, qb)
                    attn_out_block(b, qb)
            else:
                for qb in range(NQB):
                    proj_block(b, qb)
                for qb in range(NQB):
                    attn_out_block(b, qb)

        # ---- sum the 16 heads across cores; core c keeps rows c*512.. ----
        rs_out = dram.tile([512, D], f32, tag="rs")
        nc.gpsimd.collective_compute(
            "ReduceScatter", mybir.AluOpType.add, replica_groups=G8,
            ins=[partial.opt()], outs=[rs_out[:].opt()])
        for i in range(4):
            tf = sb_fin.tile([P, D], f32, tag="finf", name=f"tf{i}")
            nc.gpsimd.dma_start(out=tf, in_=rs_out[i * P:(i + 1) * P, :])
            tb_ = sb_fin.tile([P, D], bf, tag="finb", name=f"tb{i}")
            nc.vector.tensor_copy(tb_, tf)
            eng = nc.sync if i % 2 == 0 else nc.scalar
            eng.dma_start(out=out[i * P:(i + 1) * P, :], in_=tb_)

    nc.compile()
    return nc


# ---------------------------------------------------------------------------
# host runner: cached jit over 8 axon-tunneled cores
# ---------------------------------------------------------------------------

class _Runner:
    def __init__(self, causal: bool):
        import jax
        from jax.sharding import Mesh, PartitionSpec, NamedSharding
        try:
            from jax import shard_map
            def _shard_map(f, mesh, in_specs, out_specs):
                return shard_map(f, mesh=mesh, in_specs=in_specs,
                                 out_specs=out_specs, check_vma=False)
        except Exception:
            from jax.experimental.shard_map import shard_map
            def _shard_map(f, mesh, in_specs, out_specs):
                return shard_map(f, mesh=mesh, in_specs=in_specs,
                                 out_specs=out_specs, check_rep=False)
        from concourse.bass2jax import (_bass_exec_p, install_neuronx_cc_hook,
                                        partition_id_tensor)
        install_neuronx_cc_hook()

        self.jax = jax
        nc = _build(causal)
        self.nc = nc
        partition_name = (nc.partition_id_tensor.name
                          if nc.partition_id_tensor else None)
        in_names, out_names, out_avals = [], [], []
        for alloc in nc.m.functions[0].allocations:
            if not isinstance(alloc, mybir.MemoryLocationSet):
                continue
            name = alloc.memorylocations[0].name
            if alloc.kind == "ExternalInput":
                if name != partition_name:
                    in_names.append(name)
            elif alloc.kind == "ExternalOutput":
                out_names.append(name)
                out_avals.append(jax.core.ShapedArray(
                    tuple(alloc.tensor_shape), mybir.dt.np(alloc.dtype)))
        assert in_names == ["blob"] and out_names == ["out"], (in_names,
                                                               out_names)
        self.out_avals = out_avals
        all_names = (in_names + out_names
                     + ([partition_name] if partition_name else []))

        def _body(*args):
            operands = list(args)
            if partition_name is not None:
                operands.append(partition_id_tensor())
            outs = _bass_exec_p.bind(
                *operands, out_avals=tuple(out_avals),
                in_names=tuple(all_names), out_names=tuple(out_names),
                lowering_input_output_aliases=(),
                sim_require_finite=True, sim_require_nnan=True, nc=nc)
            return tuple(outs)

        devices = jax.devices()[:NC8]
        assert len(devices) == NC8, f"need {NC8} devices, have {len(devices)}"
        mesh = Mesh(np.asarray(devices), ("core",))
        self.sharding = NamedSharding(mesh, PartitionSpec("core"))
        self.sharded = jax.jit(_shard_map(
            _body, mesh, (PartitionSpec("core"),) * 2,
            (PartitionSpec("core"),)))
        # donation placeholder for the ExternalOutput tensor; the kernel
        # writes every element, so it stays resident and is never re-sent
        av = out_avals[0]
        self.zeros_dev = jax.device_put(
            np.zeros((NC8 * av.shape[0], *av.shape[1:]), av.dtype),
            self.sharding)

    def run(self, blob: np.ndarray) -> np.ndarray:
        outs = self.sharded(blob, self.zeros_dev)
        return np.asarray(outs[0])


_CACHE = {}


def _get_runner(causal: bool) -> _Runner:
    if causal not in _CACHE:
        _CACHE[causal] = _Runner(causal)
    return _CACHE[causal]


_CANON_MASK = None


def _is_causal(mask: np.ndarray) -> bool:
    global _CANON_MASK
    if mask.shape != (T, T) or mask.dtype != np.float32:
        return False
    if _CANON_MASK is None:
        _CANON_MASK = np.where(np.tril(np.ones((T, T), dtype=bool)),
                               np.float32(0.0),
                               np.float32(-np.inf)).astype(np.float32)
    return np.array_equal(mask, _CANON_MASK)


def _bf16_to_f32(a: np.ndarray) -> np.ndarray:
    return a.astype(np.float32)


def _make_blob(x, freqs_cos, freqs_sin, mask, wq, wk, wv, wo, causal):
    n = NBLOB if causal else NBLOB_NC
    blob = np.empty((NC8, n), BF)
    # x natural rows, cast during assignment
    blob[:, OX:OCS] = np.asarray(x, np.float32).reshape(NC8, 512 * T)
    cos_e = np.repeat(np.asarray(freqs_cos, np.float32).T, 2, axis=0)
    sin_e = np.repeat(np.asarray(freqs_sin, np.float32).T, 2, axis=0)
    cs = np.concatenate([cos_e, sin_e], axis=0)  # [256, T] fp32
    blob[:, OCS:OWQ] = cs.reshape(NC8, 32 * T)
    blob[:, OWQ:OWK] = np.asarray(wq, np.float32).reshape(
        D, NC8, HPC * HD).transpose(1, 0, 2).reshape(NC8, D * HPC * HD)
    blob[:, OWK:OWV] = np.asarray(wk, np.float32).reshape(
        2, 1024, HK, HD).transpose(2, 0, 1, 3).reshape(NC8, 1024 * HD)
    blob[:, OWV:OWO] = np.asarray(wv, np.float32).reshape(
        2, 1024, HK, HD).transpose(2, 0, 1, 3).reshape(NC8, 1024 * HD)
    blob[:, OWO:NBLOB] = np.asarray(wo, np.float32).reshape(
        NC8, HPC * HD * D)
    if not causal:
        mk = np.asarray(mask, np.float32).T / np.float32(SCALE)
        blob[:, OMK:NBLOB_NC] = mk.reshape(NC8, 256 * T)
    return blob


def kernel(x, freqs_cos, freqs_sin, mask, wq, wk, wv, wo):
    mask = np.asarray(mask, np.float32)
    causal = _is_causal(mask)
    r = _get_runner(causal)
    blob = _make_blob(x, freqs_cos, freqs_sin, mask, wq, wk, wv, wo, causal)
    res = r.run(blob.reshape(NC8 * 1, -1))  # global [(8*1), N]
    return _bf16_to_f32(res).reshape(B, T, D)
